# revision 6
# baseline (speedup 1.0000x reference)
"""Trainium2 Bass kernel for causal MHA (RoPE) — nn_MultiHeadAttention_84447646974458.

Sharding: 8 cores = 2 batches x 4 head-groups (tensor-parallel over heads).
Core c handles batch b=c//4, head group g=c%4 (heads 4g..4g+3).

v3 schedule: ALL projections run first (DMA-paced startup), then attention
tiles in REVERSE order (3,2,1,0) so the expensive tile-3 gather pipeline
(AllGather + readback) drains mid-kernel and the o_proj chains — whose
inputs arrive tile-by-tile — fill the whole back half; the kernel tail is
pure o_proj matmul work with every dependency long satisfied. Two-head
AllGathers (8 CC ops) fire as each head PAIR is normalized. Scores for
full (off-diagonal) key chunks are computed two-per-PSUM-pair and
exponentiated in ONE [128,1024] ACT op (the 352-cycle ACT overhead was
the attention-phase bottleneck); the softmax denominator is a GpSimd
partition_all_reduce on the DVE-accumulated probs (frees the two PSUM
banks the score pairs need and retires the ones/broadcast matmuls).
RoPE runs all-bf16 on DVE after one PSUM->SBUF downcast.
Host reassembles out[b, :, 512g:512(g+1)] from core (b,g).
"""
import math
import numpy as np
import ml_dtypes

import concourse.bass as bass
import concourse.tile as tile
from concourse import bacc, mybir
from concourse.bass_isa import ReduceOp
from concourse.bass_utils import run_bass_kernel_spmd

F32 = mybir.dt.float32
BF16 = mybir.dt.bfloat16

B, S, H = 2, 2048, 2048
NH, DH = 16, 128
HPG = 4            # heads per group (per core)
OCG = HPG * DH     # 512 channels per group
NC = 8
SCALE = 1.0 / math.sqrt(DH)
THETA = 10000.0

TQ = 512           # query-token tile (free dim of attention matmuls)
KC = H // 128      # 16 contraction chunks of 128


def _rope_tables(s):
    invf = 1.0 / (THETA ** (np.arange(0, DH, 2, dtype=np.float32) / DH))
    t = np.arange(s, dtype=np.float32)
    fr = np.concatenate([np.outer(t, invf)] * 2, axis=1)  # [s, DH]
    cosT = np.cos(fr).T.copy()                            # [DH, s]
    ssinT = np.sin(fr).T.copy()
    ssinT[:DH // 2] *= -1.0       # sign of rotate-half folded into the table
    return cosT, ssinT


def build_nc(s=S, num_devices=NC, groups=None):
    n_tq = s // TQ
    nc = bacc.Bacc("TRN2", target_bir_lowering=False, debug=False,
                   num_devices=num_devices)

    hidT = nc.dram_tensor("hidT", [H, s], BF16, kind="ExternalInput")
    wqT = nc.dram_tensor("wqT", [H, OCG], BF16, kind="ExternalInput")
    wkT = nc.dram_tensor("wkT", [H, OCG], BF16, kind="ExternalInput")
    wvT = nc.dram_tensor("wvT", [H, OCG], BF16, kind="ExternalInput")
    woT = nc.dram_tensor("woT", [H, OCG], BF16, kind="ExternalInput")
    out = nc.dram_tensor("out", [s, OCG], BF16, kind="ExternalOutput")

    # ---- host-computed constants (embedded in NEFF) ----
    cosT, ssinT = _rope_tables(s)
    # causal triangle mask tri[r, c] = (r <= c): a diagonal 128-key chunk
    # against a 512-query block is [all-zero cols | this triangle | all-one
    # cols], so only a [128,128] block ever needs a mask multiply.
    maskM = (np.arange(128)[:, None] <= np.arange(128)[None, :]).astype(np.float32)

    cosT_d = nc.inline_tensor(cosT.astype(ml_dtypes.bfloat16), name="cosT")
    ssinT_d = nc.inline_tensor(ssinT.astype(ml_dtypes.bfloat16), name="ssinT")
    maskM_d = nc.inline_tensor(maskM.astype(ml_dtypes.bfloat16), name="maskM")

    if groups is None:
        groups = [[0, 1, 2, 3], [4, 5, 6, 7]] if num_devices == 8 else [list(range(num_devices))]
    n_group = len(groups[0])

    with tile.TileContext(nc) as tc:
        with (
            tc.tile_pool(name="consts", bufs=1) as pc,
            tc.tile_pool(name="weights", bufs=1) as pw,
            tc.tile_pool(name="hid", bufs=1) as ph,
            tc.tile_pool(name="acts", bufs=1) as pa,
            tc.tile_pool(name="work", bufs=1) as pk,
            tc.tile_pool(name="probs", bufs=1) as pp,
            tc.tile_pool(name="psum", bufs=1, space="PSUM") as ps,
            tc.tile_pool(name="dram", bufs=1, space="DRAM") as pd,
        ):
            # ---- input loads, ordered to match first-use. qk_tile0(wq)
            # streams wq chunk-by-chunk against hid[:, 0:512]; wk follows so
            # the k chains never wait; the rest arrives behind.
            wq_sb = pw.tile([128, KC * OCG], BF16, tag="w", bufs=3, name="wq")
            wk_sb = pw.tile([128, KC * OCG], BF16, tag="w", bufs=3, name="wk")
            wv_sb = pw.tile([128, KC * OCG], BF16, tag="w", bufs=3, name="wv")
            hid_sb = [ph.tile([128, s], BF16, name=f"hid{hh}", tag="hid",
                              bufs=KC) for hh in range(KC)]
            q1 = s // 4
            for hh in range(KC):
                nc.sync.dma_start(wq_sb[:, hh * OCG:(hh + 1) * OCG],
                                  wqT[hh * 128:(hh + 1) * 128, :])
                nc.sync.dma_start(hid_sb[hh][:, 0:q1],
                                  hidT[hh * 128:(hh + 1) * 128, 0:q1])
            # rope tables ride the (idle) Activation DMA queue; emitted after
            # the wq/hid stream so they don't compete for HBM at t=0
            cos_sb = pc.tile([DH, s], BF16)
            nc.scalar.dma_start(cos_sb[:], cosT_d[:])
            ssin_sb = pc.tile([DH, s], BF16)
            nc.scalar.dma_start(ssin_sb[:], ssinT_d[:])
            for hh in range(KC):
                nc.sync.dma_start(wk_sb[:, hh * OCG:(hh + 1) * OCG],
                                  wkT[hh * 128:(hh + 1) * 128, :])
            for hh in range(KC):
                nc.sync.dma_start(hid_sb[hh][:, q1:2 * q1],
                                  hidT[hh * 128:(hh + 1) * 128, q1:2 * q1])
            for hh in range(KC):
                nc.sync.dma_start(wv_sb[:, hh * OCG:(hh + 1) * OCG],
                                  wvT[hh * 128:(hh + 1) * 128, :])
            for hh in range(KC):
                nc.sync.dma_start(hid_sb[hh][:, 2 * q1:s],
                                  hidT[hh * 128:(hh + 1) * 128, 2 * q1:s])
            mask_sb = pc.tile([128, 128], BF16)
            nc.sync.dma_start(mask_sb[:], maskM_d[:])

            qT_sb = pa.tile([128, HPG * s], BF16, name="qT")
            kT_sb = pa.tile([128, HPG * s], BF16, name="kT")
            v_sb = pa.tile([128, (s // 128) * OCG], BF16, name="v")

            def rope(pm, dst, tq):
                """dst[:, :TQ] = pm*cos + rot_half(pm)*ssin, all-bf16 on DVE
                (2x/4x modes) after one PSUM->SBUF downcast; rotate-half's
                sign lives in the ssin table."""
                c0, c1 = tq * TQ, (tq + 1) * TQ
                pmb = pk.tile([128, TQ], BF16, tag="ra", bufs=2)
                nc.vector.tensor_copy(pmb[:], pm[:])
                rot = pk.tile([128, TQ], BF16, tag="rr", bufs=2)
                nc.vector.tensor_copy(rot[0:64, :], pmb[64:128, :])
                nc.vector.tensor_copy(rot[64:128, :], pmb[0:64, :])
                a = pk.tile([128, TQ], BF16, tag="rb", bufs=2)
                nc.vector.tensor_mul(a[:], pmb[:], cos_sb[:, c0:c1])
                b = pk.tile([128, TQ], BF16, tag="rs", bufs=2)
                nc.vector.tensor_mul(b[:], rot[:], ssin_sb[:, c0:c1])
                nc.vector.tensor_add(dst, a[:], b[:])

            def qk_tile(w_sb, dst_sb, j, tq):
                """One [128ch x 512tok] projection chain + RoPE (j-outer)."""
                pm = ps.tile([128, TQ], F32, tag="mm", bufs=2)
                for hh in range(KC):
                    nc.tensor.matmul(
                        pm[:],
                        w_sb[:, hh * OCG + j * 128: hh * OCG + (j + 1) * 128],
                        hid_sb[hh][:, tq * TQ:(tq + 1) * TQ],
                        start=(hh == 0), stop=(hh == KC - 1))
                rope(pm, dst_sb[:, j * s + tq * TQ: j * s + (tq + 1) * TQ], tq)

            def qk_tile0(w_sb, dst_sb):
                """tq=0 projection with contraction outer (DMA pacing)."""
                pms = [ps.tile([128, TQ], F32, tag=t, bufs=2, name=f"p0{i}")
                       for i, t in enumerate(("mm", "mm", "o", "o"))]
                for hh in range(KC):
                    for j in range(HPG):
                        nc.tensor.matmul(
                            pms[j][:],
                            w_sb[:, hh * OCG + j * 128: hh * OCG + (j + 1) * 128],
                            hid_sb[hh][:, 0:TQ],
                            start=(hh == 0), stop=(hh == KC - 1))
                        if hh == KC - 1:
                            rope(pms[j], dst_sb[:, j * s: j * s + TQ], 0)

            def v_tile(tcch):
                pm = ps.tile([128, OCG], F32, tag="o", bufs=2)
                for hh in range(KC):
                    nc.tensor.matmul(
                        pm[:],
                        hid_sb[hh][:, tcch * 128:(tcch + 1) * 128],
                        wv_sb[:, hh * OCG:(hh + 1) * OCG],
                        start=(hh == 0), stop=(hh == KC - 1))
                nc.scalar.activation(v_sb[:, tcch * OCG:(tcch + 1) * OCG], pm[:],
                                     mybir.ActivationFunctionType.Copy)

            def qk(tq):
                if tq == 0:
                    qk_tile0(wq_sb, qT_sb)
                    qk_tile0(wk_sb, kT_sb)
                else:
                    for j in range(HPG):
                        qk_tile(wq_sb, qT_sb, j, tq)
                    for j in range(HPG):
                        qk_tile(wk_sb, kT_sb, j, tq)

            def vproj(tq):
                for sub in range(TQ // 128):
                    v_tile(tq * (TQ // 128) + sub)

            # ====== per-PAIR AllGather + readback bookkeeping ======
            # ag_in[t][p]: this core's heads 2p,2p+1 of tile t, [256, TQ].
            # ag_out[t][p]: [4*256, TQ] — group-rank g's slice at rows
            # g*256 + j*128 = global head 4g+2p+j. Readback lands in the
            # recycled hid slots so at_sb[t][h][:, g*TQ:(g+1)*TQ] holds
            # global head 4g+h of tile t.
            NP = HPG // 2
            ag_in = [[pd.tile([2 * 128, TQ], BF16, name=f"agi{t}_{p}",
                              tag="agi", bufs=n_tq * NP) for p in range(NP)]
                     for t in range(n_tq)]
            ag_out = [[pd.tile([n_group * 2 * 128, TQ], BF16,
                               name=f"ago{t}_{p}", tag="ago",
                               bufs=n_tq * NP) for p in range(NP)]
                      for t in range(n_tq)]
            at_sb = [[None] * HPG for _ in range(n_tq)]

            def readback(t, p):
                """ag_out[t][p] -> recycled hid slots (WAR: free once the
                projections are done; all are, by any attention tile)."""
                for j in range(2):
                    h = 2 * p + j
                    rt = ph.tile([128, n_group * TQ], BF16, tag="hid",
                                 bufs=KC, name=f"at{t}_{h}")
                    for g in range(n_group):
                        nc.sync.dma_start(
                            rt[:, g * TQ:(g + 1) * TQ],
                            ag_out[t][p][g * 256 + j * 128:
                                         g * 256 + (j + 1) * 128, :])
                    at_sb[t][h] = rt

            def norm_head(po, sb, hd, tq):
                """ot = po * (1/sum) — the sum is already broadcast across
                partitions by GpSimd; po is read straight from PSUM by the
                DVE mul. The pair's AllGather fires after the odd head."""
                recipf = pk.tile([128, TQ], F32, tag="rc", bufs=2)
                nc.vector.reciprocal_approx_fast(recipf[:], sb[:])
                ot = pk.tile([128, TQ], BF16, tag="ot", bufs=2 * HPG,
                             name=f"ot{tq}_{hd}")
                nc.vector.tensor_mul(ot[:], po[:], recipf[:])
                p = hd // 2
                nc.scalar.dma_start(
                    ag_in[tq][p][(hd % 2) * 128:(hd % 2 + 1) * 128, :], ot[:])
                if hd % 2 == 1:
                    nc.gpsimd.collective_compute(
                        "AllGather", mybir.AluOpType.bypass,
                        replica_groups=groups,
                        ins=[ag_in[tq][p][:].opt()],
                        outs=[ag_out[tq][p][:].opt()])

            def attention(tq, fillers=(), per_pair=None):
                nkk = HPG * (tq + 1)
                nfull = HPG * tq
                lag = []
                fill_iter = iter(fillers)

                def produce(unit):
                    """Score matmul(s) + exp for one unit (a pair of full
                    key chunks sharing one [128,1024] exp, or a single
                    diagonal chunk). Diagonal chunk j: query cols < 128j
                    are fully masked — never computed; the triangular
                    128-col block gets a DVE mask multiply."""
                    st = ps.tile([128, 2 * TQ], F32, tag="st", bufs=2)
                    pr = pp.tile([128, 2 * TQ], BF16, tag="pr", bufs=3)
                    if unit[0] == 'P':
                        kks = unit[1:]
                        for half, kk in enumerate(kks):
                            nc.tensor.matmul(
                                st[:, half * TQ:(half + 1) * TQ],
                                kT_sb[:, hd * s + kk * 128:
                                      hd * s + (kk + 1) * 128],
                                qT_sb[:, hd * s + tq * TQ:
                                      hd * s + (tq + 1) * TQ],
                                start=True, stop=True)
                        nc.scalar.activation(pr[:], st[:],
                                             mybir.ActivationFunctionType.Exp,
                                             scale=SCALE)
                        return [(kk, pr, half * TQ, 0)
                                for half, kk in enumerate(kks)]
                    kk = unit[1]
                    j = kk - nfull
                    c0 = 128 * j
                    nc.tensor.matmul(
                        st[:, c0:TQ],
                        kT_sb[:, hd * s + kk * 128: hd * s + (kk + 1) * 128],
                        qT_sb[:, hd * s + tq * TQ + c0: hd * s + (tq + 1) * TQ],
                        start=True, stop=True)
                    nc.scalar.activation(pr[:, c0:TQ], st[:, c0:TQ],
                                         mybir.ActivationFunctionType.Exp,
                                         scale=SCALE)
                    nc.vector.tensor_mul(pr[:, c0:c0 + 128],
                                         pr[:, c0:c0 + 128], mask_sb[:])
                    return [(kk, pr, 0, c0)]

                units = ([('P', kk, kk + 1) for kk in range(0, nfull, 2)] +
                         [('D', kk) for kk in range(nfull, nkk)])

                for hd in range(HPG):
                    po = ps.tile([128, TQ], F32, tag="o", bufs=2)
                    # Probs accumulate on DVE (bf16); GpSimd turns acc into
                    # the partition-summed, partition-broadcast denominator.
                    acc = pk.tile([128, TQ], BF16, tag="acc", bufs=2)
                    pend = produce(units[0])
                    for ui in range(len(units)):
                        cur = pend
                        if ui + 1 < len(units):
                            pend = produce(units[ui + 1])
                        for kk, pr, base, c0 in cur:
                            nc.tensor.matmul(
                                po[:, c0:],
                                v_sb[:, kk * OCG + hd * 128:
                                     kk * OCG + (hd + 1) * 128],
                                pr[:, base + c0: base + TQ],
                                start=(kk == 0), stop=(kk == nkk - 1),
                                skip_group_check=True)
                            if kk == 0:
                                nc.vector.tensor_copy(acc[:], pr[:, 0:TQ])
                            else:
                                nc.vector.tensor_add(
                                    acc[:, c0:], acc[:, c0:],
                                    pr[:, base + c0: base + TQ])
                    sb = pk.tile([128, TQ], F32, tag="sumb", bufs=2)
                    nc.gpsimd.partition_all_reduce(sb[:], acc[:], 128,
                                                   ReduceOp.add)
                    lag.append((po, sb, hd))
                    if hd > 0:
                        p_, s_, h_ = lag.pop(0)
                        norm_head(p_, s_, h_, tq)
                        if per_pair and h_ % 2 == 1:
                            per_pair(h_ // 2)
                    f = next(fill_iter, None)
                    if f:
                        f()
                p_, s_, h_ = lag.pop(0)
                norm_head(p_, s_, h_, tq)
                if per_pair:
                    per_pair(h_ // 2)
                for f in fill_iter:
                    f()

            # ====== o_proj: column-sharded, per 128-token sub-tile ======
            def oproj_sub(t, sub):
                """out[tokens of (t,sub), own 512 cols]: 16-chunk contraction
                over the gathered global heads (readback layout above)."""
                pm = ps.tile([128, OCG], F32, tag="mm", bufs=2)
                for gh in range(n_group * HPG):
                    g, h = gh // HPG, gh % HPG
                    src = at_sb[t][h]
                    nc.tensor.matmul(
                        pm[:],
                        src[:, g * TQ + sub * 128: g * TQ + (sub + 1) * 128],
                        wo_sb[:, gh * OCG:(gh + 1) * OCG],
                        start=(gh == 0), stop=(gh == n_group * HPG - 1))
                ob = pk.tile([128, OCG], BF16, tag="ob", bufs=6)
                nc.vector.tensor_copy(ob[:], pm[:])
                r0 = t * TQ + sub * 128
                # Activation HWDGE queue: the sync queue tail may hold
                # readback triggers still waiting on their AllGather.
                nc.scalar.dma_start(out[r0:r0 + 128, :], ob[:])

            # ================= schedule =================
            qk(0)
            qk(1)
            vproj(0)
            vproj(1)
            qk(2)
            vproj(2)
            qk(3)
            vproj(3)
            # wo reuses wq's SBUF slot; triggers ride the Activation queue so
            # their wait (wq's last reader = qk(3)) can't block other DMAs.
            wo_sb = pw.tile([128, KC * OCG], BF16, tag="w", bufs=3, name="wo")
            for hh in range(KC):
                nc.scalar.dma_start(wo_sb[:, hh * OCG:(hh + 1) * OCG],
                                    woT[hh * 128:(hh + 1) * 128, :])
            # Attention tiles big-to-small: tile 3's gathers complete while
            # tiles 2..0 run; o_proj(t) slots in as readback(t) lands.
            attention(3)
            attention(2, per_pair=lambda p: readback(3, p))
            attention(1, fillers=[lambda: oproj_sub(3, 0),
                                  lambda: oproj_sub(3, 1),
                                  lambda: oproj_sub(3, 2),
                                  lambda: oproj_sub(3, 3)],
                      per_pair=lambda p: readback(2, p))
            attention(0, fillers=[lambda: oproj_sub(2, 0),
                                  lambda: oproj_sub(2, 1),
                                  lambda: oproj_sub(2, 2),
                                  lambda: oproj_sub(2, 3)],
                      per_pair=lambda p: readback(1, p))
            for p in range(NP):
                readback(0, p)
            for sub in range(TQ // 128):
                oproj_sub(1, sub)
            for sub in range(TQ // 128):
                oproj_sub(0, sub)

    nc.compile()
    return nc


_NC_CACHE = {}


def _get_nc():
    if "nc" not in _NC_CACHE:
        _NC_CACHE["nc"] = build_nc()
    return _NC_CACHE["nc"]


def _build_in_maps(hidden_states, w_qkv, w_o):
    bf = ml_dtypes.bfloat16
    hidT = [np.ascontiguousarray(hidden_states[b].T).astype(bf) for b in range(B)]
    wq_all = w_qkv[:H].T.astype(bf)
    wk_all = w_qkv[H:2 * H].T.astype(bf)
    wv_all = w_qkv[2 * H:].T.astype(bf)
    wo_all = w_o.T.astype(bf)
    in_maps = []
    for c in range(NC):
        b, g = c // 4, c % 4
        sl = slice(g * OCG, (g + 1) * OCG)
        in_maps.append({
            "hidT": hidT[b],
            "wqT": np.ascontiguousarray(wq_all[:, sl]),
            "wkT": np.ascontiguousarray(wk_all[:, sl]),
            "wvT": np.ascontiguousarray(wv_all[:, sl]),
            "woT": np.ascontiguousarray(wo_all[:, sl]),
        })
    return in_maps


def kernel(hidden_states, w_qkv, w_o):
    hidden_states = np.asarray(hidden_states, dtype=np.float32)
    w_qkv = np.asarray(w_qkv, dtype=np.float32)
    w_o = np.asarray(w_o, dtype=np.float32)

    nc = _get_nc()
    in_maps = _build_in_maps(hidden_states, w_qkv, w_o)
    res = run_bass_kernel_spmd(nc, in_maps, core_ids=list(range(NC)))

    out = np.empty((B, S, H), np.float32)
    for c in range(NC):
        b, g = c // 4, c % 4
        out[b, :, g * OCG:(g + 1) * OCG] = \
            np.asarray(res.results[c]["out"], dtype=np.float32)
    return out
